# revision 11
# baseline (speedup 1.0000x reference)
"""Trainium2 Bass kernel for nn_Attention_15771119911478 (RBF attention w/ RoPE).

Sharding: core h (of 8) computes head h for both batches (packed on partition
halves). Per-core output is the head's contribution to out @ Wo.T in [s, e]
layout, minus a per-row factor exp(-g*qn[s]) applied on the host. Host sums
the 8 per-core partials.

Host prep per head (cheap O(S*d^2)):
  qro = rope(q @ Wq_h.T).T          [64, S] per batch, bf16
  kro = 2g * rope(q @ Wk_h.T).T     [64, S] per batch, bf16
  vsb = (q @ W_vo) * exp(-g*kn)[:,None]  (kn bias folded in as a
                                     multiplicative factor)
Device math per core:
  scs[t,s] = exp(kro[:,t].qro[:,s])              (bias-free exp)
  out2[s,e] = sum_t scs[t,s] * vsb[t,e]

Structure (cost-model driven): work is chunked at <=512 score columns per
batch. Chunk (j, c) = strip j (128 t-rows), s in [128j+512c, ...). Chunks
are emitted in a diagonal wavefront (sorted by s-extent), so qro/kro DMA
chunks, qk matmuls, exps, sv matmuls and evacs all pipeline in one pass.
PSUM: 3 x [128, 1024] f32 qk slots (2 banks each, b-stride 512) + 2 x
[128, 512] f32 sv-output tiles (1 bank each; 4 strips' sv outputs per tile,
one merged evac per 4 strips). sv_i is emitted as soon as the wave covers
s-block i for all strips j <= i.

The exp over ~4.46M score elements is split per chunk between DVE (one-op
Schraudolph fast-exp: i16(x*A16 + bias) bitcast to bf16; for the leading
diag chunk of each strip the f32 bias tensor holds B16 off-diagonal and
-1e9 in the causal triangle, which saturates to -32768 = bf16 -0.0, so
masking is free) and ACT (exact Exp), greedily balancing modeled engine
time. Input DMAs are split between the SP and Pool (swdge) queues.
"""
import os
import sys

sys.path.insert(0, "/opt/trn_rl_repo")

import numpy as np
import ml_dtypes

S = 2048
D = 64
H = 8
B = 2
N_CORES = 8
SCALE = 1.0 / 8.0  # 1/sqrt(64)
BF16 = ml_dtypes.bfloat16

# Schraudolph fast-exp constants for direct bf16(i16) output:
# i16 = rnne(x * 128/ln2 + (127*128 - C16)); bit pattern read as bf16.
A16 = 128.0 / np.log(2.0)
C16 = 7.0
B16 = 127.0 * 128.0 - C16
MASK_NEG = -1.0e9  # saturates the i16 convert to -32768 = bf16 -0.0

_PROG = None
LAST_RESULTS = None


def _chunk_order():
    """Column-major chunk enumeration: (g, j, s0, cw, first).

    Group g covers s-column [512g, 512g+512). Strip j participates iff
    128j < 512(g+1); its chunk is [max(128j, 512g), 512g+512), `first`
    (diag-bearing, causally masked) iff 128j >= 512g.
    """
    items = []
    for g in range(4):
        for j in range(min(16, 4 * g + 4)):
            s0 = max(128 * j, 512 * g)
            cw = 512 * g + 512 - s0
            items.append((g, j, s0, cw, 128 * j >= 512 * g))
    return items


def _assign_engines():
    """Greedy per-chunk engine split, balancing modeled ACT/DVE time.

    Returns (plan, evac_eng):
      plan[(g, j)] = wd  (DVE Schraudolph span [0, wd); ACT Exp [wd, cw)).
      For first-chunks the DVE span covers the diag and is masked via the
      bmask bias.
      evac_eng[k] in {"D", "A"} for the merged sv evacs.
    """
    # modeled per-op costs (ns)
    dve_el, act_el = 1.0417, 0.8333
    dve_op, act_op = 195.0, 242.0
    load = {"D": 0.0, "A": 1283.0}  # ACT starts with the exp table load
    plan = {}
    evac_eng = {}
    for g, j, s0, cw, first in _chunk_order():
        if first:
            # DVE span must cover the diag (>=128) and is masked. Before
            # the full bmask has arrived (~group 1) only wd=128 is safe
            # (the mini bmask DMA covers the triangle).
            opts = [128] if g == 0 else [128, 256, 384, 512]
            opts = sorted({min(w, cw) for w in opts})
        else:
            opts = [0, cw // 2, cw]
        best, bw = None, None
        for wd in opts:
            d = load["D"] + (2 * wd * dve_el + dve_op if wd > 0 else 0.0)
            a = load["A"] + (2 * (cw - wd) * act_el + act_op if wd < cw else 0.0)
            m = max(d, a)
            if best is None or m < best:
                best, bw = m, wd
        wd = bw
        plan[(g, j)] = wd
        if wd > 0:
            load["D"] += 2 * wd * dve_el + dve_op
        if wd < cw:
            load["A"] += 2 * (cw - wd) * act_el + act_op
        if j == 4 * g + 3 or (g, j) == (3, 15):
            if load["D"] + 728 <= load["A"] + 669:
                evac_eng[g] = "D"
                load["D"] += 728
            else:
                evac_eng[g] = "A"
                load["A"] += 669
    return plan, evac_eng


def _build_program():
    import concourse.bass as bass
    import concourse.bacc as bacc
    import concourse.tile as tile
    from concourse import mybir

    f32 = mybir.dt.float32
    bf16 = mybir.dt.bfloat16
    i16 = mybir.dt.int16
    Exp = mybir.ActivationFunctionType.Exp
    MULT = mybir.AluOpType.mult
    ADD = mybir.AluOpType.add

    plan, evac_eng = _assign_engines()

    nc = bacc.Bacc(
        "TRN2",
        target_bir_lowering=False,
        debug=False,
        enable_asserts=False,
        num_devices=N_CORES,
    )

    def din(name, shape, dt):
        return nc.dram_tensor(name, shape, dt, kind="ExternalInput").ap()

    t_bm = din("bmask", [128, 1024], f32)  # Schraudolph bias w/ causal mask
    t_qro = din("qro", [128, S], bf16)
    t_kro = din("kro", [128, S], bf16)
    t_vsb = din("vsb", [128, 2 * 1024], bf16)  # vsb per batch
    t_out = nc.dram_tensor("out", [128, S], f32, kind="ExternalOutput").ap()

    def Wj(j):
        return 2048 - 128 * j

    with tile.TileContext(nc) as tc:
        with (
            tc.tile_pool(name="const", bufs=1) as const,
            tc.tile_pool(name="big", bufs=1) as big,
            tc.tile_pool(name="scp", bufs=1) as scp,
            tc.tile_pool(name="pp", bufs=3, space="PSUM") as pp,
            tc.tile_pool(name="svp", bufs=2, space="PSUM") as svp,
        ):
            # ---- SBUF tiles ----
            bmask = const.tile([128, 1024], f32, tag="bmask")
            bm3 = bmask.rearrange("p (b c) -> p b c", b=2)  # [128, 2, 512]
            qro = big.tile([128, S], bf16, tag="qro")
            kro = big.tile([128, S], bf16, tag="kro")
            vsbt = big.tile([128, 2 * 1024], bf16, tag="vsbt")
            vsb = [vsbt[:, 0:1024], vsbt[:, 1024:2048]]
            outsb = big.tile([128, S], f32, tag="outsb")
            scs, scs3 = {}, {}
            for j in range(16):
                scs[j] = scp.tile(
                    [128, 2 * Wj(j)], bf16, tag=f"sc_{j}", name=f"sc_{j}"
                )
                scs3[j] = scs[j].rearrange("p (b c) -> p b c", b=2)

            def slot():
                return pp.tile([128, 1024], f32, tag="slot", name="slot")

            def svslot():
                return svp.tile([128, 512], f32, tag="svg", name="svg")

            # ---- input DMAs. SP: critical low chunks + bmask triangle
            # minis; Pool (swdge): the rest, in need order.
            nc.sync.dma_start(kro[:, 0:512], t_kro[:, 0:512])
            nc.sync.dma_start(qro[:, 0:512], t_qro[:, 0:512])
            nc.sync.dma_start(bmask[:, 0:128], t_bm[:, 0:128])
            nc.sync.dma_start(bmask[:, 512:640], t_bm[:, 512:640])
            nc.sync.dma_start(qro[:, 512:1024], t_qro[:, 512:1024])
            nc.sync.dma_start(kro[:, 512:1024], t_kro[:, 512:1024])
            nc.gpsimd.dma_start(bmask[:, 128:512], t_bm[:, 128:512])
            nc.gpsimd.dma_start(bmask[:, 640:1024], t_bm[:, 640:1024])
            nc.gpsimd.dma_start(qro[:, 1024:1536], t_qro[:, 1024:1536])
            nc.gpsimd.dma_start(vsbt[:, 0:512], t_vsb[:, 0:512])
            nc.gpsimd.dma_start(vsbt[:, 1024:1536], t_vsb[:, 1024:1536])
            nc.gpsimd.dma_start(qro[:, 1536:2048], t_qro[:, 1536:2048])
            nc.gpsimd.dma_start(kro[:, 1024:2048], t_kro[:, 1024:2048])
            nc.gpsimd.dma_start(vsbt[:, 512:1024], t_vsb[:, 512:1024])
            nc.gpsimd.dma_start(vsbt[:, 1536:2048], t_vsb[:, 1536:2048])

            # preload ACT exp table from the first-arriving DMA chunk
            scratch = const.tile([128, 1], f32, tag="scratch")
            nc.scalar.activation(scratch[:], kro[:, 0:1], Exp)

            def emit_chunk(g, j, s0, cw, first):
                ps = slot()
                for b in (0, 1):
                    rows = slice(64 * b, 64 * b + 64)
                    tp = (0, 0) if b == 0 else (64, 0)
                    nc.tensor.matmul(
                        ps[:, b * 512 : b * 512 + cw],
                        kro[rows, j * 128 : j * 128 + 128],
                        qro[rows, s0 : s0 + cw],
                        start=True, stop=True, tile_position=tp,
                    )
                ps3 = ps.rearrange("p (b c) -> p b c", b=2)[:, :, 0:cw]
                r0 = s0 - 128 * j  # strip-relative col of the chunk start
                out3 = scs3[j][:, :, r0 : r0 + cw]
                wd = plan[(g, j)]
                if wd > 0:
                    o = out3[:, :, 0:wd].bitcast(i16)
                    if first:
                        nc.vector.scalar_tensor_tensor(
                            o, ps3[:, :, 0:wd], A16, bm3[:, :, 0:wd], MULT, ADD
                        )
                    else:
                        nc.vector.tensor_scalar(
                            o, ps3[:, :, 0:wd], A16, B16, MULT, ADD
                        )
                if wd < cw:
                    nc.scalar.activation(out3[:, :, wd:cw], ps3[:, :, wd:cw], Exp)

            svg = {}

            def emit_sv(i):
                k = i // 4
                if i % 4 == 0:
                    svg[k] = svslot()
                ps = svg[k]
                for b in (0, 1):
                    col = (i % 4) * 128 + b * 64
                    for j in range(i + 1):
                        nc.tensor.matmul(
                            ps[:, col : col + 64],
                            scs[j][:, b * Wj(j) + 128 * (i - j) :
                                   b * Wj(j) + 128 * (i - j) + 128],
                            vsb[b][:, j * 64 : j * 64 + 64],
                            start=(j == 0), stop=(j == i),
                        )

            def emit_evac(k, c0, c1):
                # evacuate sv outputs of strips 4k+c0 .. 4k+c1-1
                if evac_eng[k] == "A":
                    nc.scalar.copy(
                        outsb[:, 512 * k + 128 * c0 : 512 * k + 128 * c1],
                        svg[k][:, 128 * c0 : 128 * c1],
                    )
                else:
                    nc.vector.tensor_copy(
                        outsb[:, 512 * k + 128 * c0 : 512 * k + 128 * c1],
                        svg[k][:, 128 * c0 : 128 * c1],
                    )
                nc.gpsimd.dma_start(
                    t_out[:, 512 * k + 128 * c0 : 512 * k + 128 * c1],
                    outsb[:, 512 * k + 128 * c0 : 512 * k + 128 * c1],
                )
                if c1 == 4:
                    svg.pop(k)

            # ---- column-major sweep ----
            for g, j, s0, cw, first in _chunk_order():
                emit_chunk(g, j, s0, cw, first)
                if first:
                    emit_sv(j)  # block j is fully covered once strip j lands
                    if g < 3:
                        if j == 4 * g + 3:
                            emit_evac(g, 0, 4)
                    else:
                        # last group: drain in two pieces to shorten the tail
                        if j == 14:
                            emit_evac(3, 0, 3)
                        elif j == 15:
                            emit_evac(3, 3, 4)

    nc.compile()
    return nc


def _get_program():
    global _PROG
    if _PROG is None:
        _PROG = _build_program()
    return _PROG


def _rope_T(x):
    # interleaved RoPE on [S, 64], returns [64, S] f32
    f = np.arange(32, dtype=np.float64)
    freqs = 1.0 / (10000.0 ** (2 * f / 64))
    ang = np.arange(S, dtype=np.float64)[:, None] * freqs[None, :]
    c = np.cos(ang)
    s = np.sin(ang)
    x1, x2 = x[:, 0::2].astype(np.float64), x[:, 1::2].astype(np.float64)
    out = np.empty((S, 64), np.float64)
    out[:, 0::2] = x1 * c - x2 * s
    out[:, 1::2] = x1 * s + x2 * c
    return out.T.astype(np.float32)


def _prep_inputs(q, Wq, Wk, Wv, Wo, gamma):
    """Build the per-core in_maps (all host-side numpy)."""
    q = np.asarray(q, np.float32)
    Wq = np.asarray(Wq, np.float32)
    Wk = np.asarray(Wk, np.float32)
    Wv = np.asarray(Wv, np.float32)
    Wo = np.asarray(Wo, np.float32)
    gamma = np.asarray(gamma, np.float32)

    # Schraudolph bias tile [128, 2*512] f32: per-batch halves; triangle
    # (t > s masked -> -1e9) in cols 0:128 of each half, B16 elsewhere.
    bm = np.full((128, 1024), B16, np.float32)
    blocked = ~np.triu(np.ones((128, 128), bool))  # mask t > s (strictly)
    for h0 in (0, 512):
        bm[:, h0 : h0 + 128] = np.where(blocked, MASK_NEG, B16)

    in_maps = []
    qn_exp = np.zeros((B, H, S), np.float32)
    for h in range(H):
        g = float(gamma[h]) * SCALE
        Wq_h = Wq[h * 64 : (h + 1) * 64]
        Wk_h = Wk[h * 64 : (h + 1) * 64]
        Wv_h = Wv[h * 64 : (h + 1) * 64]
        Wo_h = Wo[:, h * 64 : (h + 1) * 64]  # [64(e), 64(d)]
        W_vo = Wv_h.T @ Wo_h.T  # [64(i), 64(e)] : q @ W_vo = vh @ Wo_h.T

        qro_b, kro_b, vsb_b = [], [], []
        for b in range(B):
            qh = q[b] @ Wq_h.T
            kh = q[b] @ Wk_h.T
            qro_b.append(_rope_T(qh))
            kro_b.append(_rope_T(kh) * (2.0 * g))
            kn = (kh * kh).sum(-1)
            w2 = (q[b] @ W_vo) * np.exp(-g * kn)[:, None]  # [S, 64]
            vsb_b.append(
                w2.reshape(16, 128, 64).transpose(1, 0, 2).reshape(128, 1024)
            )
            qn = (qh * qh).sum(-1)
            qn_exp[b, h] = np.exp(-g * qn)

        qro = np.concatenate(qro_b, 0).astype(BF16)  # [128, S]
        kro = np.concatenate(kro_b, 0).astype(BF16)
        vsb = np.concatenate(vsb_b, 1).astype(BF16)  # [128, 2*1024]

        in_maps.append(
            {
                "bmask": np.ascontiguousarray(bm),
                "qro": np.ascontiguousarray(qro),
                "kro": np.ascontiguousarray(kro),
                "vsb": np.ascontiguousarray(vsb),
            }
        )
    return in_maps, qn_exp


def kernel(q, Wq, Wk, Wv, Wo, gamma):
    global LAST_RESULTS
    from concourse import bass_utils

    nc = _get_program()
    in_maps, qn_exp = _prep_inputs(q, Wq, Wk, Wv, Wo, gamma)
    trace = bool(int(os.environ.get("KERNEL_TRACE", "0")))
    res = bass_utils.run_bass_kernel_spmd(
        nc, in_maps, core_ids=list(range(N_CORES)), trace=trace
    )
    LAST_RESULTS = res

    final = np.zeros((B, S, D), np.float32)
    for h in range(H):
        o = np.asarray(res.results[h]["out"], np.float32)  # [128, S]
        # col block i: [b0(64) | b1(64)] for s-strip i; row r = s offset
        o4 = o.reshape(128, 16, 2, 64)  # [r, i, b, e]
        for b in range(B):
            ob = o4[:, :, b, :].transpose(1, 0, 2).reshape(S, D)  # [s, e]
            final[b] += ob * qn_exp[b, h][:, None]
    return final


# revision 15
# speedup vs baseline: 1.0067x; 1.0067x over previous
"""Trainium2 Bass kernel for nn_Attention_15771119911478 (RBF attention w/ RoPE).

Sharding: core h (of 8) computes head h for both batches (packed on partition
halves). Per-core output is the head's contribution to out @ Wo.T in [s, e]
layout, minus a per-row factor exp(-g*qn[s]) applied on the host. Host sums
the 8 per-core partials.

Host prep per head (cheap O(S*d^2)):
  qro = rope(q @ Wq_h.T).T          [64, S] per batch, bf16
  kro = 2g * rope(q @ Wk_h.T).T     [64, S] per batch, bf16
  vsb = (q @ W_vo) * exp(-g*kn)[:,None]  (kn bias folded in as a
                                     multiplicative factor)
Device math per core:
  scs[t,s] = exp(kro[:,t].qro[:,s])              (bias-free exp)
  out2[s,e] = sum_t scs[t,s] * vsb[t,e]

Structure (cost-model driven): work is chunked at <=512 score columns per
batch. Chunk (j, c) = strip j (128 t-rows), s in [128j+512c, ...). Chunks
are emitted in a diagonal wavefront (sorted by s-extent), so qro/kro DMA
chunks, qk matmuls, exps, sv matmuls and evacs all pipeline in one pass.
PSUM: 3 x [128, 1024] f32 qk slots (2 banks each, b-stride 512) + 2 x
[128, 512] f32 sv-output tiles (1 bank each; 4 strips' sv outputs per tile,
one merged evac per 4 strips). sv_i is emitted as soon as the wave covers
s-block i for all strips j <= i.

The exp over ~4.46M score elements is split per chunk between DVE (one-op
Schraudolph fast-exp: i16(x*A16 + bias) bitcast to bf16; for the leading
diag chunk of each strip the f32 bias tensor holds B16 off-diagonal and
-1e9 in the causal triangle, which saturates to -32768 = bf16 -0.0, so
masking is free) and ACT (exact Exp), greedily balancing modeled engine
time. Input DMAs are split between the SP and Pool (swdge) queues.
"""
import os
import sys

sys.path.insert(0, "/opt/trn_rl_repo")

import numpy as np
import ml_dtypes

S = 2048
D = 64
H = 8
B = 2
N_CORES = 8
SCALE = 1.0 / 8.0  # 1/sqrt(64)
BF16 = ml_dtypes.bfloat16

# Schraudolph fast-exp constants for direct bf16(i16) output:
# i16 = rnne(x * 128/ln2 + (127*128 - C16)); bit pattern read as bf16.
A16 = 128.0 / np.log(2.0)
C16 = 7.0
B16 = 127.0 * 128.0 - C16
MASK_NEG = -1.0e9  # saturates the i16 convert to -32768 = bf16 -0.0

_PROG = None
LAST_RESULTS = None


def _chunk_order():
    """Column-major chunk enumeration: (g, j, s0, cw, first).

    Group g covers s-column [512g, 512g+512). Strip j participates iff
    128j < 512(g+1); its chunk is [max(128j, 512g), 512g+512), `first`
    (diag-bearing, causally masked) iff 128j >= 512g.
    """
    items = []
    for g in range(4):
        for j in range(min(16, 4 * g + 4)):
            s0 = max(128 * j, 512 * g)
            cw = 512 * g + 512 - s0
            items.append((g, j, s0, cw, 128 * j >= 512 * g))
    return items


def _assign_engines():
    """Greedy per-chunk engine split, balancing modeled ACT/DVE time.

    Returns (plan, evac_eng):
      plan[(g, j)] = wd  (DVE Schraudolph span [0, wd); ACT Exp [wd, cw)).
      For first-chunks the DVE span covers the diag and is masked via the
      bmask bias.
      evac_eng[k] in {"D", "A"} for the merged sv evacs.
    """
    # modeled per-op costs (ns)
    dve_el, act_el = 1.0417, 0.8333
    dve_op, act_op = 195.0, 242.0
    load = {"D": 0.0, "A": 1283.0}  # ACT starts with the exp table load
    plan = {}
    evac_eng = {}
    last = ["S"]
    for g, j, s0, cw, first in _chunk_order():
        if first:
            # DVE span must cover the diag (>=128) and is masked. Before
            # the full bmask has arrived (~group 1) only wd=128 is safe
            # (the mini bmask DMA covers the triangle).
            opts = [128] if g == 0 else [128, 256, 384, 512]
            opts = sorted({min(w, cw) for w in opts})
        else:
            opts = [0, cw // 2, cw]
        best, bw = None, None
        for wd in opts:
            d = load["D"] + (2 * wd * dve_el + dve_op if wd > 0 else 0.0)
            a = load["A"] + (2 * (cw - wd) * act_el + act_op if wd < cw else 0.0)
            # smoothness bias: penalize giving this chunk to the same
            # single engine as the previous chunk (keeps both engines fed)
            m = max(d, a)
            if wd == 0 and last[0] == "A":
                m += 300.0
            if wd == cw and last[0] == "D":
                m += 300.0
            if best is None or m < best:
                best, bw = m, wd
        wd = bw
        last[0] = "D" if wd == cw else ("A" if wd == 0 else "S")
        plan[(g, j)] = wd
        if wd > 0:
            load["D"] += 2 * wd * dve_el + dve_op
        if wd < cw:
            load["A"] += 2 * (cw - wd) * act_el + act_op
        if j == 4 * g + 3 or (g, j) == (3, 15):
            if load["D"] + 728 <= load["A"] + 669:
                evac_eng[g] = "D"
                load["D"] += 728
            else:
                evac_eng[g] = "A"
                load["A"] += 669
    return plan, evac_eng


def _build_program():
    import concourse.bass as bass
    import concourse.bacc as bacc
    import concourse.tile as tile
    from concourse import mybir

    f32 = mybir.dt.float32
    bf16 = mybir.dt.bfloat16
    i16 = mybir.dt.int16
    Exp = mybir.ActivationFunctionType.Exp
    MULT = mybir.AluOpType.mult
    ADD = mybir.AluOpType.add

    plan, evac_eng = _assign_engines()

    nc = bacc.Bacc(
        "TRN2",
        target_bir_lowering=False,
        debug=False,
        enable_asserts=False,
        num_devices=N_CORES,
    )

    def din(name, shape, dt):
        return nc.dram_tensor(name, shape, dt, kind="ExternalInput").ap()

    t_bm = din("bmask", [128, 1024], f32)  # Schraudolph bias w/ causal mask
    t_qro = din("qro", [128, S], bf16)
    t_kro = din("kro", [128, S], bf16)
    t_vsb = din("vsb", [128, 2 * 1024], bf16)  # vsb per batch
    t_out = nc.dram_tensor("out", [128, S], f32, kind="ExternalOutput").ap()

    def Wj(j):
        return 2048 - 128 * j

    with tile.TileContext(nc) as tc:
        with (
            tc.tile_pool(name="const", bufs=1) as const,
            tc.tile_pool(name="big", bufs=1) as big,
            tc.tile_pool(name="scp", bufs=1) as scp,
            tc.tile_pool(name="pp", bufs=3, space="PSUM") as pp,
            tc.tile_pool(name="svp", bufs=2, space="PSUM") as svp,
        ):
            # ---- SBUF tiles ----
            bmask = const.tile([128, 1024], f32, tag="bmask")
            bm3 = bmask.rearrange("p (b c) -> p b c", b=2)  # [128, 2, 512]
            qro = big.tile([128, S], bf16, tag="qro")
            kro = big.tile([128, S], bf16, tag="kro")
            vsbt = big.tile([128, 2 * 1024], bf16, tag="vsbt")
            vsb = [vsbt[:, 0:1024], vsbt[:, 1024:2048]]
            outsb = big.tile([128, S], f32, tag="outsb")
            scs, scs3 = {}, {}
            for j in range(16):
                scs[j] = scp.tile(
                    [128, 2 * Wj(j)], bf16, tag=f"sc_{j}", name=f"sc_{j}"
                )
                scs3[j] = scs[j].rearrange("p (b c) -> p b c", b=2)

            def slot():
                return pp.tile([128, 1024], f32, tag="slot", name="slot")

            def svslot():
                return svp.tile([128, 512], f32, tag="svg", name="svg")

            # ---- input DMAs. SP: critical low chunks + bmask triangle
            # minis; Pool (swdge): the rest, in need order.
            nc.sync.dma_start(kro[:, 0:128], t_kro[:, 0:128])
            nc.sync.dma_start(qro[:, 0:512], t_qro[:, 0:512])
            nc.sync.dma_start(kro[:, 128:512], t_kro[:, 128:512])
            nc.sync.dma_start(bmask[:, 0:128], t_bm[:, 0:128])
            nc.sync.dma_start(bmask[:, 512:640], t_bm[:, 512:640])
            nc.sync.dma_start(qro[:, 512:1024], t_qro[:, 512:1024])
            nc.sync.dma_start(kro[:, 512:1024], t_kro[:, 512:1024])
            nc.gpsimd.dma_start(bmask[:, 128:512], t_bm[:, 128:512])
            nc.gpsimd.dma_start(bmask[:, 640:1024], t_bm[:, 640:1024])
            nc.gpsimd.dma_start(qro[:, 1024:1536], t_qro[:, 1024:1536])
            nc.gpsimd.dma_start(vsbt[:, 0:512], t_vsb[:, 0:512])
            nc.gpsimd.dma_start(vsbt[:, 1024:1536], t_vsb[:, 1024:1536])
            nc.gpsimd.dma_start(qro[:, 1536:2048], t_qro[:, 1536:2048])
            nc.gpsimd.dma_start(kro[:, 1024:2048], t_kro[:, 1024:2048])
            nc.gpsimd.dma_start(vsbt[:, 512:1024], t_vsb[:, 512:1024])
            nc.gpsimd.dma_start(vsbt[:, 1536:2048], t_vsb[:, 1536:2048])

            # preload ACT exp table from the first-arriving DMA chunk
            scratch = const.tile([128, 1], f32, tag="scratch")
            nc.scalar.activation(scratch[:], kro[:, 0:1], Exp)

            def emit_chunk(g, j, s0, cw, first):
                ps = slot()
                for b in (0, 1):
                    rows = slice(64 * b, 64 * b + 64)
                    tp = (0, 0) if b == 0 else (64, 0)
                    nc.tensor.matmul(
                        ps[:, b * 512 : b * 512 + cw],
                        kro[rows, j * 128 : j * 128 + 128],
                        qro[rows, s0 : s0 + cw],
                        start=True, stop=True, tile_position=tp,
                    )
                ps3 = ps.rearrange("p (b c) -> p b c", b=2)[:, :, 0:cw]
                r0 = s0 - 128 * j  # strip-relative col of the chunk start
                out3 = scs3[j][:, :, r0 : r0 + cw]
                wd = plan[(g, j)]
                if wd > 0:
                    o = out3[:, :, 0:wd].bitcast(i16)
                    if first:
                        nc.vector.scalar_tensor_tensor(
                            o, ps3[:, :, 0:wd], A16, bm3[:, :, 0:wd], MULT, ADD
                        )
                    else:
                        nc.vector.tensor_scalar(
                            o, ps3[:, :, 0:wd], A16, B16, MULT, ADD
                        )
                if wd < cw:
                    nc.scalar.activation(out3[:, :, wd:cw], ps3[:, :, wd:cw], Exp)

            svg = {}

            def emit_sv(i):
                k = i // 4
                if i % 4 == 0:
                    svg[k] = svslot()
                ps = svg[k]
                for b in (0, 1):
                    col = (i % 4) * 128 + b * 64
                    for j in range(i + 1):
                        nc.tensor.matmul(
                            ps[:, col : col + 64],
                            scs[j][:, b * Wj(j) + 128 * (i - j) :
                                   b * Wj(j) + 128 * (i - j) + 128],
                            vsb[b][:, j * 64 : j * 64 + 64],
                            start=(j == 0), stop=(j == i),
                        )

            def emit_evac(k, c0, c1):
                # evacuate sv outputs of strips 4k+c0 .. 4k+c1-1
                if evac_eng[k] == "A":
                    nc.scalar.copy(
                        outsb[:, 512 * k + 128 * c0 : 512 * k + 128 * c1],
                        svg[k][:, 128 * c0 : 128 * c1],
                    )
                else:
                    nc.vector.tensor_copy(
                        outsb[:, 512 * k + 128 * c0 : 512 * k + 128 * c1],
                        svg[k][:, 128 * c0 : 128 * c1],
                    )
                eng = nc.sync if k == 3 else nc.gpsimd
                eng.dma_start(
                    t_out[:, 512 * k + 128 * c0 : 512 * k + 128 * c1],
                    outsb[:, 512 * k + 128 * c0 : 512 * k + 128 * c1],
                )
                if c1 == 4:
                    svg.pop(k)

            # ---- column-major sweep ----
            for g, j, s0, cw, first in _chunk_order():
                emit_chunk(g, j, s0, cw, first)
                if first:
                    emit_sv(j)  # block j is fully covered once strip j lands
                    if g < 3:
                        if j == 4 * g + 3:
                            emit_evac(g, 0, 4)
                    else:
                        # last group: drain in two pieces to shorten the tail
                        if j == 14:
                            emit_evac(3, 0, 3)
                        elif j == 15:
                            emit_evac(3, 3, 4)

    nc.compile()
    return nc


def _get_program():
    global _PROG
    if _PROG is None:
        _PROG = _build_program()
    return _PROG


def _rope_T(x):
    # interleaved RoPE on [S, 64], returns [64, S] f32
    f = np.arange(32, dtype=np.float64)
    freqs = 1.0 / (10000.0 ** (2 * f / 64))
    ang = np.arange(S, dtype=np.float64)[:, None] * freqs[None, :]
    c = np.cos(ang)
    s = np.sin(ang)
    x1, x2 = x[:, 0::2].astype(np.float64), x[:, 1::2].astype(np.float64)
    out = np.empty((S, 64), np.float64)
    out[:, 0::2] = x1 * c - x2 * s
    out[:, 1::2] = x1 * s + x2 * c
    return out.T.astype(np.float32)


def _prep_inputs(q, Wq, Wk, Wv, Wo, gamma):
    """Build the per-core in_maps (all host-side numpy)."""
    q = np.asarray(q, np.float32)
    Wq = np.asarray(Wq, np.float32)
    Wk = np.asarray(Wk, np.float32)
    Wv = np.asarray(Wv, np.float32)
    Wo = np.asarray(Wo, np.float32)
    gamma = np.asarray(gamma, np.float32)

    # Schraudolph bias tile [128, 2*512] f32: per-batch halves; triangle
    # (t > s masked -> -1e9) in cols 0:128 of each half, B16 elsewhere.
    bm = np.full((128, 1024), B16, np.float32)
    blocked = ~np.triu(np.ones((128, 128), bool))  # mask t > s (strictly)
    for h0 in (0, 512):
        bm[:, h0 : h0 + 128] = np.where(blocked, MASK_NEG, B16)

    in_maps = []
    qn_exp = np.zeros((B, H, S), np.float32)
    for h in range(H):
        g = float(gamma[h]) * SCALE
        Wq_h = Wq[h * 64 : (h + 1) * 64]
        Wk_h = Wk[h * 64 : (h + 1) * 64]
        Wv_h = Wv[h * 64 : (h + 1) * 64]
        Wo_h = Wo[:, h * 64 : (h + 1) * 64]  # [64(e), 64(d)]
        W_vo = Wv_h.T @ Wo_h.T  # [64(i), 64(e)] : q @ W_vo = vh @ Wo_h.T

        qro_b, kro_b, vsb_b = [], [], []
        for b in range(B):
            qh = q[b] @ Wq_h.T
            kh = q[b] @ Wk_h.T
            qro_b.append(_rope_T(qh))
            kro_b.append(_rope_T(kh) * (2.0 * g))
            kn = (kh * kh).sum(-1)
            w2 = (q[b] @ W_vo) * np.exp(-g * kn)[:, None]  # [S, 64]
            vsb_b.append(
                w2.reshape(16, 128, 64).transpose(1, 0, 2).reshape(128, 1024)
            )
            qn = (qh * qh).sum(-1)
            qn_exp[b, h] = np.exp(-g * qn)

        qro = np.concatenate(qro_b, 0).astype(BF16)  # [128, S]
        kro = np.concatenate(kro_b, 0).astype(BF16)
        vsb = np.concatenate(vsb_b, 1).astype(BF16)  # [128, 2*1024]

        in_maps.append(
            {
                "bmask": np.ascontiguousarray(bm),
                "qro": np.ascontiguousarray(qro),
                "kro": np.ascontiguousarray(kro),
                "vsb": np.ascontiguousarray(vsb),
            }
        )
    return in_maps, qn_exp


def kernel(q, Wq, Wk, Wv, Wo, gamma):
    global LAST_RESULTS
    from concourse import bass_utils

    nc = _get_program()
    in_maps, qn_exp = _prep_inputs(q, Wq, Wk, Wv, Wo, gamma)
    trace = bool(int(os.environ.get("KERNEL_TRACE", "0")))
    res = bass_utils.run_bass_kernel_spmd(
        nc, in_maps, core_ids=list(range(N_CORES)), trace=trace
    )
    LAST_RESULTS = res

    final = np.zeros((B, S, D), np.float32)
    for h in range(H):
        o = np.asarray(res.results[h]["out"], np.float32)  # [128, S]
        # col block i: [b0(64) | b1(64)] for s-strip i; row r = s offset
        o4 = o.reshape(128, 16, 2, 64)  # [r, i, b, e]
        for b in range(B):
            ob = o4[:, :, b, :].transpose(1, 0, 2).reshape(S, D)  # [s, e]
            final[b] += ob * qn_exp[b, h][:, None]
    return final


# revision 16
# speedup vs baseline: 1.0200x; 1.0132x over previous
"""Trainium2 Bass kernel for nn_Attention_15771119911478 (RBF attention w/ RoPE).

Sharding: core h (of 8) computes head h for both batches (packed on partition
halves). Per-core output is the head's contribution to out @ Wo.T in [s, e]
layout, minus a per-row factor exp(-g*qn[s]) applied on the host. Host sums
the 8 per-core partials.

Host prep per head (cheap O(S*d^2)):
  qro = rope(q @ Wq_h.T).T          [64, S] per batch, bf16
  kro = 2g * rope(q @ Wk_h.T).T     [64, S] per batch, bf16
  vsb = (q @ W_vo) * exp(-g*kn)[:,None]  (kn bias folded in as a
                                     multiplicative factor)
Device math per core:
  scs[t,s] = exp(kro[:,t].qro[:,s])              (bias-free exp)
  out2[s,e] = sum_t scs[t,s] * vsb[t,e]

Structure (cost-model driven): work is chunked at <=512 score columns per
batch. Chunk (j, c) = strip j (128 t-rows), s in [128j+512c, ...). Chunks
are emitted in a diagonal wavefront (sorted by s-extent), so qro/kro DMA
chunks, qk matmuls, exps, sv matmuls and evacs all pipeline in one pass.
PSUM: 3 x [128, 1024] f32 qk slots (2 banks each, b-stride 512) + 2 x
[128, 512] f32 sv-output tiles (1 bank each; 4 strips' sv outputs per tile,
one merged evac per 4 strips). sv_i is emitted as soon as the wave covers
s-block i for all strips j <= i.

The exp over ~4.46M score elements is split per chunk between DVE (one-op
Schraudolph fast-exp: i16(x*A16 + bias) bitcast to bf16; for the leading
diag chunk of each strip the f32 bias tensor holds B16 off-diagonal and
-1e9 in the causal triangle, which saturates to -32768 = bf16 -0.0, so
masking is free) and ACT (exact Exp), greedily balancing modeled engine
time. Input DMAs are split between the SP and Pool (swdge) queues.
"""
import os
import sys

sys.path.insert(0, "/opt/trn_rl_repo")

import numpy as np
import ml_dtypes

S = 2048
D = 64
H = 8
B = 2
N_CORES = 8
SCALE = 1.0 / 8.0  # 1/sqrt(64)
BF16 = ml_dtypes.bfloat16

# Schraudolph fast-exp constants for direct bf16(i16) output:
# i16 = rnne(x * 128/ln2 + (127*128 - C16)); bit pattern read as bf16.
A16 = 128.0 / np.log(2.0)
C16 = 7.0
B16 = 127.0 * 128.0 - C16
MASK_NEG = -1.0e9  # saturates the i16 convert to -32768 = bf16 -0.0

_PROG = None
LAST_RESULTS = None


def _chunk_order():
    """Column-major chunk enumeration: (g, j, s0, cw, first).

    Group g covers s-column [512g, 512g+512). Strip j participates iff
    128j < 512(g+1); its chunk is [max(128j, 512g), 512g+512), `first`
    (diag-bearing, causally masked) iff 128j >= 512g.
    """
    items = []
    for g in range(4):
        for j in range(min(16, 4 * g + 4)):
            s0 = max(128 * j, 512 * g)
            cw = 512 * g + 512 - s0
            items.append((g, j, s0, cw, 128 * j >= 512 * g))
    return items


def _assign_engines():
    """Greedy per-chunk engine split, balancing modeled ACT/DVE time.

    Returns (plan, evac_eng):
      plan[(g, j)] = wd  (DVE Schraudolph span [0, wd); ACT Exp [wd, cw)).
      For first-chunks the DVE span covers the diag and is masked via the
      bmask bias.
      evac_eng[k] in {"D", "A"} for the merged sv evacs.
    """
    # modeled per-op costs (ns)
    dve_el, act_el = 1.0417, 0.8333
    dve_op, act_op = 195.0, 242.0
    load = {"D": 0.0, "A": 1283.0}  # ACT starts with the exp table load
    plan = {}
    evac_eng = {}
    last = ["S"]
    for g, j, s0, cw, first in _chunk_order():
        if first:
            # diag chunk: DVE does the masked 128-col diag block, ACT the
            # rest (bmask tile only needs the [128, 2, 128] triangle)
            opts = [128]
        else:
            opts = [0, cw // 2, cw]
        best, bw = None, None
        for wd in opts:
            d = load["D"] + (2 * wd * dve_el + dve_op if wd > 0 else 0.0)
            a = load["A"] + (2 * (cw - wd) * act_el + act_op if wd < cw else 0.0)
            # smoothness bias: penalize giving this chunk to the same
            # single engine as the previous chunk (keeps both engines fed)
            m = max(d, a)
            if wd == 0 and last[0] == "A":
                m += 300.0
            if wd == cw and last[0] == "D":
                m += 300.0
            if best is None or m < best:
                best, bw = m, wd
        wd = bw
        last[0] = "D" if wd == cw else ("A" if wd == 0 else "S")
        plan[(g, j)] = wd
        if wd > 0:
            load["D"] += 2 * wd * dve_el + dve_op
        if wd < cw:
            load["A"] += 2 * (cw - wd) * act_el + act_op
        if j == 4 * g + 3 or (g, j) == (3, 15):
            if load["D"] + 728 <= load["A"] + 669:
                evac_eng[g] = "D"
                load["D"] += 728
            else:
                evac_eng[g] = "A"
                load["A"] += 669
    return plan, evac_eng


def _build_program():
    import concourse.bass as bass
    import concourse.bacc as bacc
    import concourse.tile as tile
    from concourse import mybir

    f32 = mybir.dt.float32
    bf16 = mybir.dt.bfloat16
    i16 = mybir.dt.int16
    Exp = mybir.ActivationFunctionType.Exp
    MULT = mybir.AluOpType.mult
    ADD = mybir.AluOpType.add

    plan, evac_eng = _assign_engines()

    nc = bacc.Bacc(
        "TRN2",
        target_bir_lowering=False,
        debug=False,
        enable_asserts=False,
        num_devices=N_CORES,
    )

    def din(name, shape, dt):
        return nc.dram_tensor(name, shape, dt, kind="ExternalInput").ap()

    t_bm = din("bmask", [128, 256], f32)  # Schraudolph bias w/ causal mask
    t_qro = din("qro", [128, S], bf16)
    t_kro = din("kro", [128, S], bf16)
    t_vsb = din("vsb", [128, 2 * 1024], bf16)  # vsb per batch
    t_out = nc.dram_tensor("out", [128, S], f32, kind="ExternalOutput").ap()

    def Wj(j):
        return 2048 - 128 * j

    with tile.TileContext(nc) as tc:
        with (
            tc.tile_pool(name="const", bufs=1) as const,
            tc.tile_pool(name="big", bufs=1) as big,
            tc.tile_pool(name="scp", bufs=1) as scp,
            tc.tile_pool(name="pp", bufs=3, space="PSUM") as pp,
            tc.tile_pool(name="svp", bufs=2, space="PSUM") as svp,
        ):
            # ---- SBUF tiles ----
            bmask = const.tile([128, 256], f32, tag="bmask")
            bm3 = bmask.rearrange("p (b c) -> p b c", b=2)  # [128, 2, 128]
            qro = big.tile([128, S], bf16, tag="qro")
            kro = big.tile([128, S], bf16, tag="kro")
            vsbt = big.tile([128, 2 * 1024], bf16, tag="vsbt")
            vsb = [vsbt[:, 0:1024], vsbt[:, 1024:2048]]
            outsb = big.tile([128, S], f32, tag="outsb")
            scs, scs3 = {}, {}
            for j in range(16):
                scs[j] = scp.tile(
                    [128, 2 * Wj(j)], bf16, tag=f"sc_{j}", name=f"sc_{j}"
                )
                scs3[j] = scs[j].rearrange("p (b c) -> p b c", b=2)

            def slot():
                return pp.tile([128, 1024], f32, tag="slot", name="slot")

            def svslot():
                return svp.tile([128, 512], f32, tag="svg", name="svg")

            # ---- input DMAs. SP: critical low chunks + bmask triangle
            # minis; Pool (swdge): the rest, in need order.
            nc.sync.dma_start(kro[:, 0:128], t_kro[:, 0:128])
            nc.sync.dma_start(qro[:, 0:512], t_qro[:, 0:512])
            nc.sync.dma_start(bmask[:], t_bm[:])
            nc.sync.dma_start(kro[:, 128:512], t_kro[:, 128:512])
            nc.sync.dma_start(qro[:, 512:1024], t_qro[:, 512:1024])
            nc.sync.dma_start(kro[:, 512:1024], t_kro[:, 512:1024])
            nc.gpsimd.dma_start(qro[:, 1024:1536], t_qro[:, 1024:1536])
            nc.gpsimd.dma_start(vsbt[:, 0:512], t_vsb[:, 0:512])
            nc.gpsimd.dma_start(vsbt[:, 1024:1536], t_vsb[:, 1024:1536])
            nc.gpsimd.dma_start(qro[:, 1536:2048], t_qro[:, 1536:2048])
            nc.gpsimd.dma_start(kro[:, 1024:2048], t_kro[:, 1024:2048])
            nc.gpsimd.dma_start(vsbt[:, 512:1024], t_vsb[:, 512:1024])
            nc.gpsimd.dma_start(vsbt[:, 1536:2048], t_vsb[:, 1536:2048])

            # preload ACT exp table from the first-arriving DMA chunk
            scratch = const.tile([128, 1], f32, tag="scratch")
            nc.scalar.activation(scratch[:], kro[:, 0:1], Exp)

            def emit_chunk(g, j, s0, cw, first):
                ps = slot()
                for b in (0, 1):
                    rows = slice(64 * b, 64 * b + 64)
                    tp = (0, 0) if b == 0 else (64, 0)
                    nc.tensor.matmul(
                        ps[:, b * 512 : b * 512 + cw],
                        kro[rows, j * 128 : j * 128 + 128],
                        qro[rows, s0 : s0 + cw],
                        start=True, stop=True, tile_position=tp,
                    )
                ps3 = ps.rearrange("p (b c) -> p b c", b=2)[:, :, 0:cw]
                r0 = s0 - 128 * j  # strip-relative col of the chunk start
                out3 = scs3[j][:, :, r0 : r0 + cw]
                wd = plan[(g, j)]
                if wd > 0:
                    o = out3[:, :, 0:wd].bitcast(i16)
                    if first:
                        nc.vector.scalar_tensor_tensor(
                            o, ps3[:, :, 0:wd], A16, bm3[:, :, 0:wd], MULT, ADD
                        )
                    else:
                        nc.vector.tensor_scalar(
                            o, ps3[:, :, 0:wd], A16, B16, MULT, ADD
                        )
                if wd < cw:
                    nc.scalar.activation(out3[:, :, wd:cw], ps3[:, :, wd:cw], Exp)

            svg = {}

            def emit_sv(i):
                k = i // 4
                if i % 4 == 0:
                    svg[k] = svslot()
                ps = svg[k]
                for b in (0, 1):
                    col = (i % 4) * 128 + b * 64
                    for j in range(i + 1):
                        nc.tensor.matmul(
                            ps[:, col : col + 64],
                            scs[j][:, b * Wj(j) + 128 * (i - j) :
                                   b * Wj(j) + 128 * (i - j) + 128],
                            vsb[b][:, j * 64 : j * 64 + 64],
                            start=(j == 0), stop=(j == i),
                        )

            def emit_evac(k, c0, c1):
                # evacuate sv outputs of strips 4k+c0 .. 4k+c1-1
                if evac_eng[k] == "A":
                    nc.scalar.copy(
                        outsb[:, 512 * k + 128 * c0 : 512 * k + 128 * c1],
                        svg[k][:, 128 * c0 : 128 * c1],
                    )
                else:
                    nc.vector.tensor_copy(
                        outsb[:, 512 * k + 128 * c0 : 512 * k + 128 * c1],
                        svg[k][:, 128 * c0 : 128 * c1],
                    )
                eng = nc.sync if k == 3 else nc.gpsimd
                eng.dma_start(
                    t_out[:, 512 * k + 128 * c0 : 512 * k + 128 * c1],
                    outsb[:, 512 * k + 128 * c0 : 512 * k + 128 * c1],
                )
                if c1 == 4:
                    svg.pop(k)

            # ---- column-major sweep ----
            for g, j, s0, cw, first in _chunk_order():
                emit_chunk(g, j, s0, cw, first)
                if first:
                    emit_sv(j)  # block j is fully covered once strip j lands
                    if g < 3:
                        if j == 4 * g + 3:
                            emit_evac(g, 0, 4)
                    else:
                        # last group: drain in two pieces to shorten the tail
                        if j == 14:
                            emit_evac(3, 0, 3)
                        elif j == 15:
                            emit_evac(3, 3, 4)

    nc.compile()
    return nc


def _get_program():
    global _PROG
    if _PROG is None:
        _PROG = _build_program()
    return _PROG


def _rope_T(x):
    # interleaved RoPE on [S, 64], returns [64, S] f32
    f = np.arange(32, dtype=np.float64)
    freqs = 1.0 / (10000.0 ** (2 * f / 64))
    ang = np.arange(S, dtype=np.float64)[:, None] * freqs[None, :]
    c = np.cos(ang)
    s = np.sin(ang)
    x1, x2 = x[:, 0::2].astype(np.float64), x[:, 1::2].astype(np.float64)
    out = np.empty((S, 64), np.float64)
    out[:, 0::2] = x1 * c - x2 * s
    out[:, 1::2] = x1 * s + x2 * c
    return out.T.astype(np.float32)


def _prep_inputs(q, Wq, Wk, Wv, Wo, gamma):
    """Build the per-core in_maps (all host-side numpy)."""
    q = np.asarray(q, np.float32)
    Wq = np.asarray(Wq, np.float32)
    Wk = np.asarray(Wk, np.float32)
    Wv = np.asarray(Wv, np.float32)
    Wo = np.asarray(Wo, np.float32)
    gamma = np.asarray(gamma, np.float32)

    # Schraudolph bias tile [128, 2*512] f32: per-batch halves; triangle
    # (t > s masked -> -1e9) in cols 0:128 of each half, B16 elsewhere.
    bm = np.full((128, 256), B16, np.float32)
    blocked = ~np.triu(np.ones((128, 128), bool))  # mask t > s (strictly)
    for h0 in (0, 128):
        bm[:, h0 : h0 + 128] = np.where(blocked, MASK_NEG, B16)

    in_maps = []
    qn_exp = np.zeros((B, H, S), np.float32)
    for h in range(H):
        g = float(gamma[h]) * SCALE
        Wq_h = Wq[h * 64 : (h + 1) * 64]
        Wk_h = Wk[h * 64 : (h + 1) * 64]
        Wv_h = Wv[h * 64 : (h + 1) * 64]
        Wo_h = Wo[:, h * 64 : (h + 1) * 64]  # [64(e), 64(d)]
        W_vo = Wv_h.T @ Wo_h.T  # [64(i), 64(e)] : q @ W_vo = vh @ Wo_h.T

        qro_b, kro_b, vsb_b = [], [], []
        for b in range(B):
            qh = q[b] @ Wq_h.T
            kh = q[b] @ Wk_h.T
            qro_b.append(_rope_T(qh))
            kro_b.append(_rope_T(kh) * (2.0 * g))
            kn = (kh * kh).sum(-1)
            w2 = (q[b] @ W_vo) * np.exp(-g * kn)[:, None]  # [S, 64]
            vsb_b.append(
                w2.reshape(16, 128, 64).transpose(1, 0, 2).reshape(128, 1024)
            )
            qn = (qh * qh).sum(-1)
            qn_exp[b, h] = np.exp(-g * qn)

        qro = np.concatenate(qro_b, 0).astype(BF16)  # [128, S]
        kro = np.concatenate(kro_b, 0).astype(BF16)
        vsb = np.concatenate(vsb_b, 1).astype(BF16)  # [128, 2*1024]

        in_maps.append(
            {
                "bmask": np.ascontiguousarray(bm),
                "qro": np.ascontiguousarray(qro),
                "kro": np.ascontiguousarray(kro),
                "vsb": np.ascontiguousarray(vsb),
            }
        )
    return in_maps, qn_exp


def kernel(q, Wq, Wk, Wv, Wo, gamma):
    global LAST_RESULTS
    from concourse import bass_utils

    nc = _get_program()
    in_maps, qn_exp = _prep_inputs(q, Wq, Wk, Wv, Wo, gamma)
    trace = bool(int(os.environ.get("KERNEL_TRACE", "0")))
    res = bass_utils.run_bass_kernel_spmd(
        nc, in_maps, core_ids=list(range(N_CORES)), trace=trace
    )
    LAST_RESULTS = res

    final = np.zeros((B, S, D), np.float32)
    for h in range(H):
        o = np.asarray(res.results[h]["out"], np.float32)  # [128, S]
        # col block i: [b0(64) | b1(64)] for s-strip i; row r = s offset
        o4 = o.reshape(128, 16, 2, 64)  # [r, i, b, e]
        for b in range(B):
            ob = o4[:, :, b, :].transpose(1, 0, 2).reshape(S, D)  # [s, e]
            final[b] += ob * qn_exp[b, h][:, None]
    return final


# revision 19
# speedup vs baseline: 1.0818x; 1.0605x over previous
"""Trainium2 Bass kernel for nn_Attention_15771119911478 (RBF attention w/ RoPE).

Sharding: core h (of 8) computes head h for both batches (packed on partition
halves). Per-core output is the head's contribution to out @ Wo.T in [s, e]
layout, minus a per-row factor exp(-g*qn[s]) applied on the host. Host sums
the 8 per-core partials.

Host prep per head (cheap O(S*d^2)):
  qro = rope(q @ Wq_h.T).T          [64, S] per batch, bf16
  kro = 2g * rope(q @ Wk_h.T).T     [64, S] per batch, bf16
  vsb = (q @ W_vo) * exp(-g*kn)[:,None]  (kn bias folded in as a
                                     multiplicative factor)
Device math per core:
  scs[t,s] = exp(kro[:,t].qro[:,s])              (bias-free exp)
  out2[s,e] = sum_t scs[t,s] * vsb[t,e]

Structure (cost-model driven): work is chunked at <=512 score columns per
batch. Chunk (j, c) = strip j (128 t-rows), s in [128j+512c, ...). Chunks
are emitted in a diagonal wavefront (sorted by s-extent), so qro/kro DMA
chunks, qk matmuls, exps, sv matmuls and evacs all pipeline in one pass.
PSUM: 3 x [128, 1024] f32 qk slots (2 banks each, b-stride 512) + 2 x
[128, 512] f32 sv-output tiles (1 bank each; 4 strips' sv outputs per tile,
one merged evac per 4 strips). sv_i is emitted as soon as the wave covers
s-block i for all strips j <= i.

The exp over ~4.46M score elements is split per chunk between DVE (one-op
Schraudolph fast-exp: i16(x*A16 + bias) bitcast to bf16; for the leading
diag chunk of each strip the f32 bias tensor holds B16 off-diagonal and
-1e9 in the causal triangle, which saturates to -32768 = bf16 -0.0, so
masking is free) and ACT (exact Exp), greedily balancing modeled engine
time. Input DMAs are split between the SP and Pool (swdge) queues.
"""
import os
import sys

sys.path.insert(0, "/opt/trn_rl_repo")

import numpy as np
import ml_dtypes

S = 2048
D = 64
H = 8
B = 2
N_CORES = 8
SCALE = 1.0 / 8.0  # 1/sqrt(64)
BF16 = ml_dtypes.bfloat16

# Schraudolph fast-exp constants for direct bf16(i16) output:
# i16 = rnne(x * 128/ln2 + (127*128 - C16)); bit pattern read as bf16.
A16 = 128.0 / np.log(2.0)
C16 = 7.0
B16 = 127.0 * 128.0 - C16
MASK_NEG = -1.0e9  # saturates the i16 convert to -32768 = bf16 -0.0

_PROG = None
LAST_RESULTS = None


def _chunk_order():
    """Column-major chunk enumeration: (g, j, s0, cw, first).

    Group g covers s-column [512g, 512g+512). Strip j participates iff
    128j < 512(g+1); its chunk is [max(128j, 512g), 512g+512), `first`
    (diag-bearing, causally masked) iff 128j >= 512g.
    """
    items = []
    for g in range(4):
        for j in range(min(16, 4 * g + 4)):
            s0 = max(128 * j, 512 * g)
            cw = 512 * g + 512 - s0
            items.append((g, j, s0, cw, 128 * j >= 512 * g))
    return items


def _assign_engines():
    """Greedy per-chunk engine split, balancing modeled ACT/DVE time.

    Returns (plan, evac_eng):
      plan[(g, j)] = wd  (DVE Schraudolph span [0, wd); ACT Exp [wd, cw)).
      For first-chunks the DVE span covers the diag and is masked via the
      bmask bias.
      evac_eng[k] in {"D", "A"} for the merged sv evacs.
    """
    # modeled per-op costs (ns)
    dve_el, act_el = 1.0417, 0.8333
    dve_op, act_op = 195.0, 242.0
    load = {"D": 0.0, "A": 1283.0}  # ACT starts with the exp table load
    plan = {}
    evac_eng = {}
    last = ["S"]
    for g, j, s0, cw, first in _chunk_order():
        if first:
            # diag chunk: DVE does the masked 128-col diag block, ACT the
            # rest (bmask tile only needs the [128, 2, 128] triangle)
            opts = [128]
        else:
            opts = [0, cw // 2, cw]
        best, bw = None, None
        for wd in opts:
            d = load["D"] + (2 * wd * dve_el + dve_op if wd > 0 else 0.0)
            a = load["A"] + (2 * (cw - wd) * act_el + act_op if wd < cw else 0.0)
            # smoothness bias: penalize giving this chunk to the same
            # single engine as the previous chunk (keeps both engines fed)
            m = max(d, a)
            if wd == 0 and last[0] == "A":
                m += 300.0
            if wd == cw and last[0] == "D":
                m += 300.0
            if best is None or m < best:
                best, bw = m, wd
        wd = bw
        last[0] = "D" if wd == cw else ("A" if wd == 0 else "S")
        plan[(g, j)] = wd
        if wd > 0:
            load["D"] += 2 * wd * dve_el + dve_op
        if wd < cw:
            load["A"] += 2 * (cw - wd) * act_el + act_op
        if j == 4 * g + 3 or (g, j) == (3, 15):
            if load["D"] + 728 <= load["A"] + 669:
                evac_eng[g] = "D"
                load["D"] += 728
            else:
                evac_eng[g] = "A"
                load["A"] += 669
    return plan, evac_eng


def _build_program():
    import concourse.bass as bass
    import concourse.bacc as bacc
    import concourse.tile as tile
    from concourse import mybir

    f32 = mybir.dt.float32
    bf16 = mybir.dt.bfloat16
    i16 = mybir.dt.int16
    Exp = mybir.ActivationFunctionType.Exp
    MULT = mybir.AluOpType.mult
    ADD = mybir.AluOpType.add

    plan, evac_eng = _assign_engines()

    nc = bacc.Bacc(
        "TRN2",
        target_bir_lowering=False,
        debug=False,
        enable_asserts=False,
        num_devices=N_CORES,
    )

    def din(name, shape, dt):
        return nc.dram_tensor(name, shape, dt, kind="ExternalInput").ap()

    t_bm = din("bmask", [128, 256], f32)  # Schraudolph bias w/ causal mask
    t_qro = din("qro", [128, S], bf16)
    t_kro = din("kro", [128, S], bf16)
    t_vsb = din("vsb", [128, 2 * 1024], bf16)  # vsb per batch
    t_out = nc.dram_tensor("out", [128, S], f32, kind="ExternalOutput").ap()

    def Wj(j):
        return 2048 - 128 * j

    with tile.TileContext(nc) as tc:
        with (
            tc.tile_pool(name="const", bufs=1) as const,
            tc.tile_pool(name="big", bufs=1) as big,
            tc.tile_pool(name="scp", bufs=1) as scp,
            tc.tile_pool(name="pp", bufs=3, space="PSUM") as pp,
            tc.tile_pool(name="svp", bufs=2, space="PSUM") as svp,
        ):
            # ---- SBUF tiles ----
            bmask = const.tile([128, 256], f32, tag="bmask")
            bm3 = bmask.rearrange("p (b c) -> p b c", b=2)  # [128, 2, 128]
            qro = big.tile([128, S], bf16, tag="qro")
            kro = big.tile([128, S], bf16, tag="kro")
            vsbt = big.tile([128, 2 * 1024], bf16, tag="vsbt")
            vsb = [vsbt[:, 0:1024], vsbt[:, 1024:2048]]
            outsb = big.tile([128, S], f32, tag="outsb")
            scs, scs3 = {}, {}
            for j in range(16):
                scs[j] = scp.tile(
                    [128, 2 * Wj(j)], bf16, tag=f"sc_{j}", name=f"sc_{j}"
                )
                scs3[j] = scs[j].rearrange("p (b c) -> p b c", b=2)

            def slot():
                return pp.tile([128, 1024], f32, tag="slot", name="slot")

            def svslot():
                return svp.tile([128, 512], f32, tag="svg", name="svg")

            # ---- input DMAs. SP: critical low chunks + bmask triangle
            # minis; Pool (swdge): the rest, in need order.
            nc.sync.dma_start(kro[:, 0:128], t_kro[:, 0:128])
            nc.sync.dma_start(qro[:, 0:512], t_qro[:, 0:512])
            nc.sync.dma_start(bmask[:], t_bm[:])
            nc.sync.dma_start(kro[:, 128:512], t_kro[:, 128:512])
            nc.sync.dma_start(qro[:, 512:1024], t_qro[:, 512:1024])
            nc.sync.dma_start(kro[:, 512:1024], t_kro[:, 512:1024])
            nc.gpsimd.dma_start(qro[:, 1024:1536], t_qro[:, 1024:1536])
            nc.gpsimd.dma_start(vsbt[:, 0:512], t_vsb[:, 0:512])
            nc.gpsimd.dma_start(vsbt[:, 1024:1536], t_vsb[:, 1024:1536])
            nc.gpsimd.dma_start(qro[:, 1536:2048], t_qro[:, 1536:2048])
            nc.gpsimd.dma_start(kro[:, 1024:2048], t_kro[:, 1024:2048])
            nc.gpsimd.dma_start(vsbt[:, 512:1024], t_vsb[:, 512:1024])
            nc.gpsimd.dma_start(vsbt[:, 1536:2048], t_vsb[:, 1536:2048])

            # preload ACT exp table from the first-arriving DMA chunk
            scratch = const.tile([128, 1], f32, tag="scratch")
            nc.scalar.activation(scratch[:], kro[:, 0:1], Exp)

            def emit_chunk(g, j, s0, cw, first):
                ps = slot()
                for b in (0, 1):
                    rows = slice(64 * b, 64 * b + 64)
                    tp = (0, 0) if b == 0 else (64, 0)
                    nc.tensor.matmul(
                        ps[:, b * 512 : b * 512 + cw],
                        kro[rows, j * 128 : j * 128 + 128],
                        qro[rows, s0 : s0 + cw],
                        start=True, stop=True, tile_position=tp,
                    )
                ps3 = ps.rearrange("p (b c) -> p b c", b=2)[:, :, 0:cw]
                r0 = s0 - 128 * j  # strip-relative col of the chunk start
                out3 = scs3[j][:, :, r0 : r0 + cw]
                wd = plan[(g, j)]
                if wd > 0:
                    o = out3[:, :, 0:wd].bitcast(i16)
                    if first:
                        nc.vector.scalar_tensor_tensor(
                            o, ps3[:, :, 0:wd], A16, bm3[:, :, 0:wd], MULT, ADD
                        )
                    else:
                        nc.vector.tensor_scalar(
                            o, ps3[:, :, 0:wd], A16, B16, MULT, ADD
                        )
                if wd < cw:
                    nc.scalar.activation(out3[:, :, wd:cw], ps3[:, :, wd:cw], Exp)

            svg = {}

            def emit_sv(i, j0=0, j1=None):
                # sv matmuls for out block i over strips j0..j1-1 (psum
                # accumulation group continues across split emissions)
                k = i // 4
                if i % 4 == 0 and j0 == 0:
                    svg[k] = svslot()
                if j1 is None:
                    j1 = i + 1
                ps = svg[k]
                for b in (0, 1):
                    col = (i % 4) * 128 + b * 64
                    for j in range(j0, j1):
                        nc.tensor.matmul(
                            ps[:, col : col + 64],
                            scs[j][:, b * Wj(j) + 128 * (i - j) :
                                   b * Wj(j) + 128 * (i - j) + 128],
                            vsb[b][:, j * 64 : j * 64 + 64],
                            start=(j == 0), stop=(j == i),
                            skip_group_check=True,
                        )

            def emit_evac(k, c0, c1):
                # evacuate sv outputs of strips 4k+c0 .. 4k+c1-1
                if evac_eng[k] == "A":
                    nc.scalar.copy(
                        outsb[:, 512 * k + 128 * c0 : 512 * k + 128 * c1],
                        svg[k][:, 128 * c0 : 128 * c1],
                    )
                else:
                    nc.vector.tensor_copy(
                        outsb[:, 512 * k + 128 * c0 : 512 * k + 128 * c1],
                        svg[k][:, 128 * c0 : 128 * c1],
                    )
                eng = nc.sync if k == 3 else nc.gpsimd
                eng.dma_start(
                    t_out[:, 512 * k + 128 * c0 : 512 * k + 128 * c1],
                    outsb[:, 512 * k + 128 * c0 : 512 * k + 128 * c1],
                )
                if c1 == 4:
                    svg.pop(k)

            # ---- column-major sweep. Each group's sv/evac bursts are
            # deferred into the next group's continuation chunks so PE's
            # sv work never starves the exp engines at group boundaries.
            chunks = _chunk_order()

            def conts(g):
                return [it for it in chunks if it[0] == g and not it[4]]

            def firsts(g):
                return [it for it in chunks if it[0] == g and it[4]]

            def emit_chunks(items):
                for g, j, s0, cw, first in items:
                    emit_chunk(g, j, s0, cw, first)

            emit_chunks(firsts(0))
            for g in range(1, 3):
                cg = conts(g)
                emit_chunks(cg[:3])
                for i in range(4 * (g - 1), 4 * g):  # prev group's blocks
                    emit_sv(i)
                emit_evac(g - 1, 0, 4)
                emit_chunks(cg[3:])
                emit_chunks(firsts(g))
            cg = conts(3)
            emit_chunks(cg[:3])
            for i in range(8, 12):
                emit_sv(i)
            emit_evac(2, 0, 4)
            emit_chunks(cg[3:])
            # last group: full sv for each block right after its first-chunk,
            # evac split so the tail drains early
            for g, j, s0, cw, first in firsts(3):
                emit_chunk(g, j, s0, cw, first)
                emit_sv(j)
                if j == 14:
                    emit_evac(3, 0, 3)
                elif j == 15:
                    emit_evac(3, 3, 4)

    nc.compile()
    return nc


def _get_program():
    global _PROG
    if _PROG is None:
        _PROG = _build_program()
    return _PROG


def _rope_T(x):
    # interleaved RoPE on [S, 64], returns [64, S] f32
    f = np.arange(32, dtype=np.float64)
    freqs = 1.0 / (10000.0 ** (2 * f / 64))
    ang = np.arange(S, dtype=np.float64)[:, None] * freqs[None, :]
    c = np.cos(ang)
    s = np.sin(ang)
    x1, x2 = x[:, 0::2].astype(np.float64), x[:, 1::2].astype(np.float64)
    out = np.empty((S, 64), np.float64)
    out[:, 0::2] = x1 * c - x2 * s
    out[:, 1::2] = x1 * s + x2 * c
    return out.T.astype(np.float32)


def _prep_inputs(q, Wq, Wk, Wv, Wo, gamma):
    """Build the per-core in_maps (all host-side numpy)."""
    q = np.asarray(q, np.float32)
    Wq = np.asarray(Wq, np.float32)
    Wk = np.asarray(Wk, np.float32)
    Wv = np.asarray(Wv, np.float32)
    Wo = np.asarray(Wo, np.float32)
    gamma = np.asarray(gamma, np.float32)

    # Schraudolph bias tile [128, 2*512] f32: per-batch halves; triangle
    # (t > s masked -> -1e9) in cols 0:128 of each half, B16 elsewhere.
    bm = np.full((128, 256), B16, np.float32)
    blocked = ~np.triu(np.ones((128, 128), bool))  # mask t > s (strictly)
    for h0 in (0, 128):
        bm[:, h0 : h0 + 128] = np.where(blocked, MASK_NEG, B16)

    in_maps = []
    qn_exp = np.zeros((B, H, S), np.float32)
    for h in range(H):
        g = float(gamma[h]) * SCALE
        Wq_h = Wq[h * 64 : (h + 1) * 64]
        Wk_h = Wk[h * 64 : (h + 1) * 64]
        Wv_h = Wv[h * 64 : (h + 1) * 64]
        Wo_h = Wo[:, h * 64 : (h + 1) * 64]  # [64(e), 64(d)]
        W_vo = Wv_h.T @ Wo_h.T  # [64(i), 64(e)] : q @ W_vo = vh @ Wo_h.T

        qro_b, kro_b, vsb_b = [], [], []
        for b in range(B):
            qh = q[b] @ Wq_h.T
            kh = q[b] @ Wk_h.T
            qro_b.append(_rope_T(qh))
            kro_b.append(_rope_T(kh) * (2.0 * g))
            kn = (kh * kh).sum(-1)
            w2 = (q[b] @ W_vo) * np.exp(-g * kn)[:, None]  # [S, 64]
            vsb_b.append(
                w2.reshape(16, 128, 64).transpose(1, 0, 2).reshape(128, 1024)
            )
            qn = (qh * qh).sum(-1)
            qn_exp[b, h] = np.exp(-g * qn)

        qro = np.concatenate(qro_b, 0).astype(BF16)  # [128, S]
        kro = np.concatenate(kro_b, 0).astype(BF16)
        vsb = np.concatenate(vsb_b, 1).astype(BF16)  # [128, 2*1024]

        in_maps.append(
            {
                "bmask": np.ascontiguousarray(bm),
                "qro": np.ascontiguousarray(qro),
                "kro": np.ascontiguousarray(kro),
                "vsb": np.ascontiguousarray(vsb),
            }
        )
    return in_maps, qn_exp


def kernel(q, Wq, Wk, Wv, Wo, gamma):
    global LAST_RESULTS
    from concourse import bass_utils

    nc = _get_program()
    in_maps, qn_exp = _prep_inputs(q, Wq, Wk, Wv, Wo, gamma)
    trace = bool(int(os.environ.get("KERNEL_TRACE", "0")))
    res = bass_utils.run_bass_kernel_spmd(
        nc, in_maps, core_ids=list(range(N_CORES)), trace=trace
    )
    LAST_RESULTS = res

    final = np.zeros((B, S, D), np.float32)
    for h in range(H):
        o = np.asarray(res.results[h]["out"], np.float32)  # [128, S]
        # col block i: [b0(64) | b1(64)] for s-strip i; row r = s offset
        o4 = o.reshape(128, 16, 2, 64)  # [r, i, b, e]
        for b in range(B):
            ob = o4[:, :, b, :].transpose(1, 0, 2).reshape(S, D)  # [s, e]
            final[b] += ob * qn_exp[b, h][:, None]
    return final
